# revision 6
# baseline (speedup 1.0000x reference)
"""MoE gate kernel for Trainium2 (8 NeuronCores, data-parallel over tokens).

Computation per token t (64 experts, top-8):
    gate[t, e]  = sum_h x[t, h] * W[e, h]          (f32-accurate)
    biased      = gate + expert_bias
    top8 of biased -> idx (jax top_k tie semantics)
    weights     = sigmoid(gate[t, idx]) / sum(...)

Precision strategy: the f32 matmul on PE runs at 1/4 rate, so x and W are
split on the host into fp16 hi/lo pairs (lo pre-scaled by 2^11 to stay in
fp16 normal range).  gate = xh@Wh + 2^-11 * (xh@Wl_s + xl_s@Wh), which
reproduces f32-matmul-level accuracy (~1e-6) at full 1 cycle/row PE rate.
The lo*lo term (~2e-7) is dropped.

Layout strategy: host pre-transposes the per-core token shard to [h, t] so
the PE can consume it directly as the moving operand (contraction dim on
partitions) -- no on-device transposes of the 16 MiB activations, no
PSUM->SBUF copy traffic.  Matmul produces gate^T [64 experts, 512 tokens]
per group; two small PE transposes per 128-token tile bring biased/probs
into [tokens, experts] layout for the DVE top-8 ops (max / max_index).
"""

import numpy as np

N_CORES = 8
H = 2048          # hidden dim = contraction
E = 64            # experts
K = 8             # top-k
T_TOTAL = 16384   # 4*4096 tokens
T_CORE = T_TOTAL // N_CORES   # 2048
NG = 4            # token groups per core
GT = T_CORE // NG             # 512 tokens per group (one PSUM bank of f32)
NT = GT // 128                # 128-token tiles per group
KC = H // 128                 # 16 contraction chunks
LO_SCALE = float(2.0 ** 11)
INV_LO_SCALE = float(2.0 ** -11)

_CACHE = {}


def _build_nc():
    from contextlib import ExitStack

    import concourse.bass as bass
    import concourse.tile as tile
    from concourse import bacc, mybir

    f16 = mybir.dt.float16
    f32 = mybir.dt.float32
    u32 = mybir.dt.uint32
    Alu = mybir.AluOpType
    Act = mybir.ActivationFunctionType

    nc = bacc.Bacc(
        "TRN2", target_bir_lowering=False, debug=False, num_devices=N_CORES
    )

    # DRAM I/O (per core). x shards are transposed on host: [h, t_core].
    xht_d = nc.dram_tensor("xht", [H, T_CORE], f16, kind="ExternalInput").ap()
    xlt_d = nc.dram_tensor("xlt", [H, T_CORE], f16, kind="ExternalInput").ap()
    # W hi/lo transposed + pre-arranged on host to [128, KC*E] (4 KiB rows).
    wht_d = nc.dram_tensor("wht", [128, KC * E], f16, kind="ExternalInput").ap()
    wlt_d = nc.dram_tensor("wlt", [128, KC * E], f16, kind="ExternalInput").ap()
    bias_d = nc.dram_tensor("bias", [E, 1], f32, kind="ExternalInput").ap()
    nbias_d = nc.dram_tensor("nbias", [E, 1], f32, kind="ExternalInput").ap()
    ident_d = nc.dram_tensor("ident", [E, E], f32, kind="ExternalInput").ap()

    oidx_d = nc.dram_tensor("out_idx", [T_CORE, K], mybir.dt.int32,
                            kind="ExternalOutput").ap()
    ow_d = nc.dram_tensor("out_w", [T_CORE, K], f32, kind="ExternalOutput").ap()

    with tile.TileContext(nc) as tc, ExitStack() as ctx:
        xpool = ctx.enter_context(tc.tile_pool(name="x", bufs=1))
        wpool = ctx.enter_context(tc.tile_pool(name="w", bufs=1))
        gpool = ctx.enter_context(tc.tile_pool(name="gate", bufs=2))
        ppool = ctx.enter_context(tc.tile_pool(name="mm", bufs=2, space="PSUM"))
        tpool = ctx.enter_context(tc.tile_pool(name="tp", bufs=2, space="PSUM"))
        spool = ctx.enter_context(tc.tile_pool(name="small", bufs=3))
        stpool = ctx.enter_context(tc.tile_pool(name="stage", bufs=1))

        # constants / weights
    # W tiles: [128, KC*E] fp16; chunk k = [:, k*E:(k+1)*E]
        wh = wpool.tile([128, KC * E], f16, tag="wh")
        nc.sync.dma_start(wh[:], wht_d)
        wl = wpool.tile([128, KC * E], f16, tag="wl")
        nc.sync.dma_start(wl[:], wlt_d)
        bias = wpool.tile([E, 1], f32, tag="bias")
        nc.sync.dma_start(bias[:], bias_d)
        nbias = wpool.tile([E, 1], f32, tag="nbias")
        nc.sync.dma_start(nbias[:], nbias_d)
        ident = wpool.tile([E, E], f32, tag="ident")
        nc.sync.dma_start(ident[:], ident_d)

        # output staging for the whole core
        idx_st = stpool.tile([128, NG * NT * K], u32, tag="idxst")
        w_st = stpool.tile([128, NG * NT * K], f32, tag="wst")

        # x tiles, loaded group-major so group g can start after ~4 MiB.
        # tile (g, kk) covers k-chunks 2kk..2kk+1, columns g*GT..(g+1)*GT:
        # sbuf [128, 2*GT] fp16 ; dram rows (2kk*128 + {0..255}), cols g-slice
        xh_t = [[None] * (KC // 2) for _ in range(NG)]
        xl_t = [[None] * (KC // 2) for _ in range(NG)]
        for g in range(NG):
            for kk in range(KC // 2):
                src_h = xht_d.rearrange("(a p) t -> a p t", p=128)
                src_l = xlt_d.rearrange("(a p) t -> a p t", p=128)
                th = xpool.tile([128, 2, GT], f16, tag=f"xh{g}_{kk}")
                nc.sync.dma_start(
                    th[:],
                    src_h[2 * kk : 2 * kk + 2, :, g * GT : (g + 1) * GT]
                    .rearrange("a p t -> p a t"),
                )
                tl = xpool.tile([128, 2, GT], f16, tag=f"xl{g}_{kk}")
                nc.sync.dma_start(
                    tl[:],
                    src_l[2 * kk : 2 * kk + 2, :, g * GT : (g + 1) * GT]
                    .rearrange("a p t -> p a t"),
                )
                xh_t[g][kk] = th
                xl_t[g][kk] = tl

        for g in range(NG):
            # ---- matmul: gate^T[e, t] over this group's 512 tokens ----
            p1 = ppool.tile([E, GT], f32, tag="p1")
            p23 = ppool.tile([E, GT], f32, tag="p23")
            for k in range(KC):
                kk, c = divmod(k, 2)
                rh = xh_t[g][kk][:, c, :]
                rl = xl_t[g][kk][:, c, :]
                whk = wh[:, k * E : (k + 1) * E]
                wlk = wl[:, k * E : (k + 1) * E]
                nc.tensor.matmul(p1[:], lhsT=whk, rhs=rh,
                                 start=(k == 0), stop=(k == KC - 1))
                nc.tensor.matmul(p23[:], lhsT=wlk, rhs=rh,
                                 start=(k == 0), stop=False)
                nc.tensor.matmul(p23[:], lhsT=whk, rhs=rl,
                                 start=False, stop=(k == KC - 1))

            # ---- combine + bias + sigmoid (still [e, t] layout) ----
            comb = gpool.tile([E, GT], f32, tag="comb")
            nc.scalar.activation(comb[:], p23[:], Act.Identity,
                                 bias=bias[:, 0:1], scale=INV_LO_SCALE)
            biasedT = gpool.tile([E, GT], f32, tag="biasedT")
            nc.vector.tensor_tensor(biasedT[:], p1[:], comb[:], op=Alu.add)
            probsT = gpool.tile([E, GT], f32, tag="probsT")
            nc.scalar.activation(probsT[:], biasedT[:], Act.Sigmoid,
                                 bias=nbias[:, 0:1], scale=1.0)

            # ---- transpose to [t, e] ----
            tb = tpool.tile([128, NT * E], f32, tag="tb")
            tp = tpool.tile([128, NT * E], f32, tag="tp")
            for j in range(NT):
                nc.tensor.matmul(tb[:, j * E : (j + 1) * E],
                                 lhsT=biasedT[:, j * 128 : (j + 1) * 128],
                                 rhs=ident[:], is_transpose=True,
                                 start=(j == 0), stop=(j == NT - 1))
            for j in range(NT):
                nc.tensor.matmul(tp[:, j * E : (j + 1) * E],
                                 lhsT=probsT[:, j * 128 : (j + 1) * 128],
                                 rhs=ident[:], is_transpose=True,
                                 start=(j == 0), stop=(j == NT - 1))
            biased = gpool.tile([128, NT * E], f32, tag="biased")
            nc.vector.tensor_copy(biased[:], tb[:])
            probs = gpool.tile([128, NT * E], f32, tag="probs")
            nc.scalar.copy(probs[:], tp[:])

            # ---- top-8 per 128-token tile ----
            for j in range(NT):
                t_idx = g * NT + j
                bj = biased[:, j * E : (j + 1) * E]
                pj = probs[:, j * E : (j + 1) * E]
                bidx = idx_st[:, t_idx * K : (t_idx + 1) * K]

                b8 = spool.tile([128, K], f32, tag="b8")
                nc.vector.max(b8[:], bj)
                nc.vector.max_index(bidx, b8[:], bj)
                mask = spool.tile([128, E], f32, tag="mask")
                nc.vector.tensor_scalar(mask[:], bj, b8[:, 7:8], None,
                                        op0=Alu.is_ge)
                pmask = spool.tile([128, E], f32, tag="pmask")
                nc.vector.tensor_tensor(pmask[:], pj, mask[:], op=Alu.mult)
                p8 = spool.tile([128, K], f32, tag="p8")
                nc.vector.max(p8[:], pmask[:])
                pidx = spool.tile([128, K], u32, tag="pidx")
                nc.vector.max_index(pidx[:], p8[:], pmask[:])

                # permute p8 into biased-rank order:
                # w8[k] = sum_j p8[j] * (pidx[j] == bidx[k])
                eq = spool.tile([128, K * K], f32, tag="eq")
                nc.vector.tensor_tensor(
                    eq[:].rearrange("p (a b) -> p a b", a=K),
                    bidx.unsqueeze(2).broadcast_to((128, K, K)),
                    pidx[:].unsqueeze(1).broadcast_to((128, K, K)),
                    op=Alu.is_equal,
                )
                wmat = spool.tile([128, K * K], f32, tag="wmat")
                nc.vector.tensor_tensor(
                    wmat[:].rearrange("p (a b) -> p a b", a=K),
                    eq[:].rearrange("p (a b) -> p a b", a=K),
                    p8[:].unsqueeze(1).broadcast_to((128, K, K)),
                    op=Alu.mult,
                )
                w8 = spool.tile([128, K], f32, tag="w8")
                nc.vector.tensor_reduce(
                    w8[:], wmat[:].rearrange("p (a b) -> p a b", a=K),
                    axis=mybir.AxisListType.X, op=Alu.add,
                )
                den = spool.tile([128, 1], f32, tag="den")
                nc.vector.tensor_reduce(den[:], w8[:],
                                        axis=mybir.AxisListType.X, op=Alu.add)
                rec = spool.tile([128, 1], f32, tag="rec")
                nc.vector.reciprocal(rec[:], den[:])
                nc.vector.tensor_scalar_mul(
                    w_st[:, t_idx * K : (t_idx + 1) * K], w8[:], rec[:, 0:1]
                )

        # ---- store outputs ----
        nc.sync.dma_start(
            oidx_d.rearrange("(t p) k -> p t k", p=128),
            idx_st[:].rearrange("p (t k) -> p t k", k=K).bitcast(mybir.dt.int32),
        )
        nc.sync.dma_start(
            ow_d.rearrange("(t p) k -> p t k", p=128),
            w_st[:].rearrange("p (t k) -> p t k", k=K),
        )

    nc.compile()
    return nc


def _get_nc():
    if "nc" not in _CACHE:
        _CACHE["nc"] = _build_nc()
    return _CACHE["nc"]


def _host_prep(hidden_states, weight, expert_biases):
    x = np.asarray(hidden_states, np.float32).reshape(T_TOTAL, H)
    W = np.asarray(weight, np.float32)
    b = np.asarray(expert_biases, np.float32)

    xh = x.astype(np.float16)
    xl = ((x - xh.astype(np.float32)) * LO_SCALE).astype(np.float16)
    Wh = W.astype(np.float16)
    Wl = ((W - Wh.astype(np.float32)) * LO_SCALE).astype(np.float16)

    def arrange_w(Wm):
        # [E, H] -> transposed [H, E] -> [128, KC*E] with chunk k at cols k*E
        wt = np.ascontiguousarray(Wm.T)                # [H, E]
        return np.ascontiguousarray(
            wt.reshape(KC, 128, E).transpose(1, 0, 2).reshape(128, KC * E)
        )

    wh_a = arrange_w(Wh)
    wl_a = arrange_w(Wl)
    bias_pp = np.ascontiguousarray(b.reshape(E, 1))
    nbias_pp = np.ascontiguousarray(-bias_pp)
    ident = np.eye(E, dtype=np.float32)

    in_maps = []
    for c in range(N_CORES):
        sl = slice(c * T_CORE, (c + 1) * T_CORE)
        in_maps.append({
            "xht": np.ascontiguousarray(xh[sl].T),
            "xlt": np.ascontiguousarray(xl[sl].T),
            "wht": wh_a,
            "wlt": wl_a,
            "bias": bias_pp,
            "nbias": nbias_pp,
            "ident": ident,
        })
    return in_maps


def run(hidden_states, weight, expert_biases, trace=False, **spmd_kwargs):
    from concourse.bass_utils import run_bass_kernel_spmd

    in_maps = _host_prep(hidden_states, weight, expert_biases)
    nc = _get_nc()
    res = run_bass_kernel_spmd(
        nc, in_maps, core_ids=list(range(N_CORES)), trace=trace, **spmd_kwargs
    )
    idx = np.concatenate([r["out_idx"] for r in res.results], axis=0)
    w = np.concatenate([r["out_w"] for r in res.results], axis=0)
    idx = np.ascontiguousarray(idx.reshape(4, 4096, K), dtype=np.int32)
    w = np.ascontiguousarray(w.reshape(4, 4096, K), dtype=np.float32)
    return (idx, w), res


def kernel(**inputs):
    (idx, w), _ = run(**inputs)
    return idx, w
